# revision 18
# baseline (speedup 1.0000x reference)
"""Trainium2 Bass kernel for dense-MoE routing (8 experts, gate-weighted sum).

Math (restructured from the reference):
    gate   = softmax(x @ wg + bg)                  per token, E=8
    h      = relu(x @ W1cat + b1cat)               W1cat = w1 transposed/concat [C, E*H]
    out    = (gate-scaled h) @ W2p + gate @ B2W    W2p = w2.reshape(E*H,EO) @ wo (host-folded)
                                                   B2W = b2 @ wo + bo

Sharding: data-parallel over tokens; core i takes batch row i (4096 tokens).
All weights replicated.

Design (PE-bound problem: 1064 matmuls ~= 227us is the floor at bf16 rates;
fp8 DoubleRow is numerically dead here — e4m3 anywhere gives rel err >= 0.027
vs the 2e-2 gate, measured on the real data):
  - everything bf16 (rel err ~5.2e-3): halves x/w1 DMA/SBUF and enables FWL
    on every weight load (fp32r weights get no FWL).
  - mm2 runs w2p-stationary with hs moving at N=512 tokens — half the
    matmul+ldweights instructions of the po[token,oc] orientation.  Output
    comes out [oc_half, 128, T] and is de-transposed on host.
  - same-chunk software pipelining: chunk tn's mm2 trails its mm1 by `lag`
    k-tiles (kt-major, both oc-half PSUM groups open so consumption matches
    the 1 k-tile/hm production rate); ~2*lag leftover steps drain in the next
    chunk's first hms.  PE stream is dense end to end; the tail is ~8 MMs.
  - gate(tn+1) computed mid-chunk tn (softmax bias in ACT's Exp bias,
    denominator via gpsimd partition_all_reduce, gate broadcast via ONE
    DMA roundtrip on the SP queue — per-DMA dispatch on the ACT queue costs
    ~664ns of ACT SEQ and would delay relus).
  - consts split early/late so gb(0)'s roundtrip isn't queued behind the
    bulk loads.
"""

import numpy as np

_P = 128          # partitions
_T = 4096         # tokens per core
_TN = 512         # token chunk (mm1 moving dim)
_NTN = _T // _TN  # 8
_HM = 32          # hid k-tiles (4096 / 128)
_E = 8
_OC = 256         # output channels
_NCORES = 8

_CACHE = {}


def _build_nc(reps=1, loop=1, stagger=False, ppk=2, lag=3, gpos=12,
              psdve=True):
    import concourse.bacc as bacc
    import concourse.bass as bass
    import concourse.bass_isa as bass_isa
    import concourse.mybir as mybir
    import concourse.tile as tile

    f32 = mybir.dt.float32
    bf16 = mybir.dt.bfloat16
    AF = mybir.ActivationFunctionType
    ts = bass.ts

    nc = bacc.Bacc("TRN2", target_bir_lowering=False, debug=False)

    xT_d = nc.dram_tensor("xT", [2, _NTN, _P, _TN], bf16, kind="ExternalInput").ap()
    w1_d = nc.dram_tensor("w1s", [2, 4, _P, 1024], bf16, kind="ExternalInput").ap()
    w2p_d = nc.dram_tensor("w2ps", [_P, _HM, _OC], bf16, kind="ExternalInput").ap()
    b1_d = nc.dram_tensor("b1s", [_P, _HM], f32, kind="ExternalInput").ap()
    wg_d = nc.dram_tensor("wgs", [2, _P, _E], bf16, kind="ExternalInput").ap()
    bg_d = nc.dram_tensor("bgs", [_E, 1], f32, kind="ExternalInput").ap()
    b2w_d = nc.dram_tensor("b2ws", [_E, _OC], bf16, kind="ExternalInput").ap()
    ones_d = nc.dram_tensor("ones", [_E, 1], bf16, kind="ExternalInput").ap()
    gst_d = nc.dram_tensor("gstage", [_NTN, _E, _TN], bf16).ap()
    out_d = nc.dram_tensor("out", [2, _P, _T], f32, kind="ExternalOutput").ap()

    with tile.TileContext(nc) as tc:
        with (
            tc.tile_pool(name="const", bufs=1) as const,
            tc.tile_pool(name="hs", bufs=2) as p_hs,
            tc.tile_pool(name="gb", bufs=2) as p_gb,
            tc.tile_pool(name="gsmall", bufs=2) as p_gs,
            tc.tile_pool(name="gaten", bufs=3) as p_gn,
            tc.tile_pool(name="ob", bufs=4) as p_ob,
            tc.tile_pool(name="psum_h", bufs=4, space="PSUM") as psum_h,
            tc.tile_pool(name="psum_o", bufs=3, space="PSUM") as psum_o,
            tc.tile_pool(name="psum_g", bufs=1, space="PSUM") as psum_g,
        ):
            xT_sb = const.tile([_P, 2, _T], bf16, name="xT_sb")
            w1_sb = const.tile([_P, 2, 4096], bf16, name="w1_sb")
            w2p_sb = const.tile([_P, _HM, _OC], bf16, name="w2p_sb")
            b1_sb = const.tile([_P, _HM], f32, name="b1_sb")
            wg_sb = const.tile([_P, 2, _E], bf16, name="wg_sb")
            bg_sb = const.tile([_E, 1], f32, name="bg_sb")
            b2w_sb = const.tile([_E, _OC], bf16, name="b2w_sb")
            ones_sb = const.tile([_E, 1], bf16, name="ones_sb")

            for kc in range(2):
                nc.sync.dma_start(out=wg_sb[:, kc, :], in_=wg_d[kc])
            nc.sync.dma_start(out=bg_sb[:], in_=bg_d[:])
            nc.sync.dma_start(out=ones_sb[:], in_=ones_d[:])
            # chunk-major DRAM layout: every chunk is one linear stream.
            # Only the consts chunk 0 needs are loaded here; the rest are
            # emitted by emit_late_consts AFTER gate(0), so gb(0)'s DMA
            # roundtrip isn't queued behind ~4MB of bulk loads.
            for kc in range(2):
                nc.sync.dma_start(out=xT_sb[:, kc, ts(0, _TN)], in_=xT_d[kc, 0])
            for kc in range(2):
                nc.sync.dma_start(out=w1_sb[:, kc, ts(0, 1024)], in_=w1_d[kc, 0])
            nc.sync.dma_start(out=b1_sb[:], in_=b1_d[:])
            nc.sync.dma_start(out=w2p_sb[:], in_=w2p_d[:])
            nc.sync.dma_start(out=b2w_sb[:], in_=b2w_d[:])

            def emit_late_consts():
                for q in range(1, 4):
                    for kc in range(2):
                        nc.sync.dma_start(out=w1_sb[:, kc, ts(q, 1024)],
                                          in_=w1_d[kc, q])
                for tn in range(1, _NTN):
                    for kc in range(2):
                        nc.sync.dma_start(out=xT_sb[:, kc, ts(tn, _TN)],
                                          in_=xT_d[kc, tn])

            def emit_gate(tn):
                """Gate for chunk tn: 3 PE ops + ACT/DVE/gpsimd chain + DMA
                broadcast roundtrip.  Called one chunk AHEAD of use so gb is
                ready when chunk tn's DVE muls need it."""
                pg = psum_g.tile([_E, _TN], f32, name="pg", tag="pg")
                nc.tensor.matmul(pg[:], wg_sb[:, 0, :], xT_sb[:, 0, ts(tn, _TN)],
                                 start=True, stop=False)
                nc.tensor.matmul(pg[:], wg_sb[:, 1, :], xT_sb[:, 1, ts(tn, _TN)],
                                 start=False, stop=True)
                expu = p_gs.tile([_E, _TN], bf16, name="expu", tag="expu")
                nc.scalar.activation(expu[:], pg[:], AF.Exp, bias=bg_sb[:])
                # softmax denominator: gpsimd partition all-reduce frees the
                # PE of a ones-matmul; then reciprocal + re-broadcast.
                rc = p_gs.tile([1, _TN], f32, name="rc", tag="rc")
                if psdve:
                    s8 = p_gs.tile([_E, _TN], f32, name="s8", tag="s8")
                    nc.gpsimd.partition_all_reduce(s8[:], expu[:], _E,
                                                   bass_isa.ReduceOp.add)
                    nc.vector.reciprocal(rc[:], s8[0:1, :])
                else:
                    ps = psum_g.tile([1, _TN], f32, name="ps", tag="pg")
                    nc.tensor.matmul(ps[:], ones_sb[:], expu[:],
                                     start=True, stop=True)
                    nc.vector.reciprocal(rc[:], ps[:])
                rcb = p_gs.tile([_E, _TN], f32, name="rcb", tag="rcb")
                nc.gpsimd.partition_broadcast(rcb[:], rc[:])
                gatenb = p_gn.tile([_E, _TN], bf16, name="gatenb", tag="gatenb")
                nc.vector.tensor_mul(gatenb[:], expu[:], rcb[:])
                # gpsimd partition_broadcast needs base partition 0: DMA the 8
                # gate rows to DRAM, then broadcast-read onto all partitions.
                gb = p_gb.tile([_P, _E, _TN], bf16, name="gb", tag="gb")
                nc.sync.dma_start(out=gst_d[tn], in_=gatenb[:])
                src_bc = gst_d[tn:tn + 1, :, :].broadcast_to((_P, _E, _TN))
                nc.sync.dma_start(out=gb[:, :, :], in_=src_bc)
                return gatenb, gb

            def emit_mm1_pair(tn, hm, hs, gb):
                ph = psum_h.tile([_P, _TN], f32, name="ph", tag="ph")
                nc.tensor.matmul(ph[:], w1_sb[:, 0, ts(hm, _P)],
                                 xT_sb[:, 0, ts(tn, _TN)], start=True, stop=False)
                nc.tensor.matmul(ph[:], w1_sb[:, 1, ts(hm, _P)],
                                 xT_sb[:, 1, ts(tn, _TN)], start=False, stop=True)
                # relu(+bias) straight into bf16 hs, then scale by gate in place.
                nc.scalar.activation(hs[:, hm, :], ph[:], AF.Relu,
                                     bias=b1_sb[:, hm:hm + 1])
                nc.vector.tensor_mul(hs[:, hm, :], hs[:, hm, :], gb[:, hm // 4, :])

            def finish_mm2(tn, po, oc):
                ob = p_ob.tile([_P, _TN], f32, name="ob", tag="ob")
                nc.scalar.copy(ob[:], po[:])
                nc.sync.dma_start(out=out_d[oc, :, ts(tn, _TN)], in_=ob[:])

            def mm2_steps(tn, hs, gatenb, mm2_pos):
                """Yield single mm2 PE ops for one chunk: w2p stationary,
                hs moving (N=512 tokens), out [128 oc, 512 tok].  kt-major
                with both oc-half accumulation groups open so each hs k-tile
                is consumed at exactly the rate mm1 produces them (2 mm2
                steps per mm1 pair)."""
                po = mm2_pos
                for kt in range(_HM):
                    for oc in range(2):
                        nc.tensor.matmul(po[oc][:], w2p_sb[:, kt, ts(oc, _P)],
                                         hs[:, kt, :],
                                         start=(kt == 0), stop=False)
                        yield
                for oc in range(2):
                    nc.tensor.matmul(po[oc][:], b2w_sb[:, ts(oc, _P)],
                                     gatenb[:], start=False, stop=True)
                    finish_mm2(tn, po[oc], oc)
                    yield

            def emit_body():
                # Same-chunk interleave: chunk tn's mm2 runs `lag` k-tiles
                # behind its mm1 (hs[kt] needs PE->ACT->DVE before mm2 reads
                # it).  The ~2*lag leftover mm2 steps drain during the next
                # chunk's first hms, so the PE stream is dense end to end and
                # the tail is ~2*lag matmuls instead of a whole chunk's mm2.
                # gate(tn+1) is emitted mid-chunk so its DMA-broadcast
                # roundtrip is done before chunk tn+1 starts.
                _DONE = object()

                def advance(gen, n):
                    for _ in range(n):
                        if next(gen, _DONE) is _DONE:
                            return None
                    return gen

                leftover = None
                gate_next = emit_gate(0)
                if emit_body.late_consts is not None:
                    emit_body.late_consts()
                    emit_body.late_consts = None
                for tn in range(_NTN):
                    gatenb, gb = gate_next
                    hs = p_hs.tile([_P, _HM, _TN], bf16, name="hs", tag="hs")
                    cur = None
                    cur_started = False
                    for hm in range(_HM):
                        emit_mm1_pair(tn, hm, hs, gb)
                        if hm == gpos and tn + 1 < _NTN:
                            gate_next = emit_gate(tn + 1)
                        if leftover is not None:
                            leftover = advance(leftover, ppk + 1)
                        elif hm >= lag and (cur is not None or not cur_started):
                            if cur is None:
                                pos = [psum_o.tile([_P, _TN], f32, name="po",
                                                   tag="po") for _ in range(2)]
                                cur = mm2_steps(tn, hs, gatenb, pos)
                                cur_started = True
                            cur = advance(cur, ppk)
                    assert leftover is None, "mm2 leftover not drained in-chunk"
                    if not cur_started:
                        pos = [psum_o.tile([_P, _TN], f32, name="po",
                                           tag="po") for _ in range(2)]
                        cur = mm2_steps(tn, hs, gatenb, pos)
                    leftover = cur
                if leftover is not None:
                    for _ in leftover:
                        pass

            emit_body.late_consts = emit_late_consts
            if loop > 1:
                # late consts must load outside the loop (once)
                emit_late_consts()
                emit_body.late_consts = None
                with tc.For_i(0, loop, 1, staggered_reset=stagger):
                    for _rep in range(reps):
                        emit_body()
            else:
                for _rep in range(reps):
                    emit_body()

    nc.compile()
    return nc


def _prep_weights(w1, b1, w2, b2, wg, bg, wo, bo):
    import ml_dtypes
    f32 = np.float32
    bf = ml_dtypes.bfloat16
    w1 = np.asarray(w1, f32)
    w2 = np.asarray(w2, f32)
    wo = np.asarray(wo, f32)
    E, IN, HID = w1.shape
    w1s = np.ascontiguousarray(
        w1.transpose(1, 0, 2).reshape(IN, E * HID).reshape(2, _P, 4, 1024)
        .transpose(0, 2, 1, 3)).astype(bf)
    w2p = (w2.astype(np.float64).reshape(E * HID, -1) @ wo.astype(np.float64)).astype(f32)
    w2ps = np.ascontiguousarray(
        w2p.reshape(_HM, _P, _OC).transpose(1, 0, 2)).astype(bf)
    b1s = np.ascontiguousarray(np.asarray(b1, f32).reshape(E * HID).reshape(_HM, _P).T)
    b2ws = (np.asarray(b2, np.float64) @ wo.astype(np.float64)
            + np.asarray(bo, np.float64)).astype(f32).astype(bf)
    wgs = np.ascontiguousarray(np.asarray(wg, f32).reshape(2, _P, E)).astype(bf)
    bgs = np.asarray(bg, f32).reshape(_E, 1)
    ones = np.ones((_E, 1), bf)
    return dict(w1s=w1s, w2ps=w2ps, b1s=b1s, b2ws=b2ws, wgs=wgs, bgs=bgs, ones=ones)


def _make_in_maps(x, w1, b1, w2, b2, wg, bg, wo, bo):
    import ml_dtypes
    bf = ml_dtypes.bfloat16
    x = np.asarray(x, np.float32)
    b, n, c = x.shape
    weights = _prep_weights(w1, b1, w2, b2, wg, bg, wo, bo)
    x2d = x.reshape(b * n, c)
    in_maps = []
    for i in range(_NCORES):
        xc = x2d[i * _T:(i + 1) * _T]                       # [T, C]
        xT = np.ascontiguousarray(
            xc.T.reshape(2, _P, _NTN, _TN).transpose(0, 2, 1, 3)).astype(bf)
        in_maps.append({"xT": xT, **weights})
    return in_maps


def _run(x, w1, b1, w2, b2, wg, bg, wo, bo, trace=False):
    from concourse.bass_utils import run_bass_kernel_spmd

    if "nc" not in _CACHE:
        _CACHE["nc"] = _build_nc(1)
    nc = _CACHE["nc"]

    x = np.asarray(x, np.float32)
    b, n, c = x.shape
    in_maps = _make_in_maps(x, w1, b1, w2, b2, wg, bg, wo, bo)
    res = run_bass_kernel_spmd(nc, in_maps, list(range(_NCORES)), trace=trace)
    # out_d is [oc_half, 128, T] per core: de-transpose to [T, OC] on host
    outs = []
    for i in range(_NCORES):
        o = res.results[i]["out"]                           # [2, 128, T]
        outs.append(o.reshape(_OC, _T).T)                   # [T, OC]
    out = np.concatenate(outs, axis=0)
    return np.ascontiguousarray(out).reshape(b, n, _OC), res


def kernel(x, w1, b1, w2, b2, wg, bg, wo, bo):
    out, _ = _run(x, w1, b1, w2, b2, wg, bg, wo, bo, trace=False)
    return out


# revision 19
# speedup vs baseline: 1.0250x; 1.0250x over previous
"""Trainium2 Bass kernel for dense-MoE routing (8 experts, gate-weighted sum).

Math (restructured from the reference):
    gate   = softmax(x @ wg + bg)                  per token, E=8
    h      = relu(x @ W1cat + b1cat)               W1cat = w1 transposed/concat [C, E*H]
    out    = (gate-scaled h) @ W2p + gate @ B2W    W2p = w2.reshape(E*H,EO) @ wo (host-folded)
                                                   B2W = b2 @ wo + bo

Sharding: data-parallel over tokens; core i takes batch row i (4096 tokens).
All weights replicated.

Design (PE-bound problem: 1064 matmuls ~= 227us is the floor at bf16 rates;
fp8 DoubleRow is numerically dead here — e4m3 anywhere gives rel err >= 0.027
vs the 2e-2 gate, measured on the real data):
  - everything bf16 (rel err ~5.2e-3): halves x/w1 DMA/SBUF and enables FWL
    on every weight load (fp32r weights get no FWL).
  - mm2 runs w2p-stationary with hs moving at N=512 tokens — half the
    matmul+ldweights instructions of the po[token,oc] orientation.  Output
    comes out [oc_half, 128, T] and is de-transposed on host.
  - same-chunk software pipelining: chunk tn's mm2 trails its mm1 by `lag`
    k-tiles (kt-major, both oc-half PSUM groups open so consumption matches
    the 1 k-tile/hm production rate); ~2*lag leftover steps drain in the next
    chunk's first hms.  PE stream is dense end to end; the tail is ~8 MMs.
  - gate(tn+1) computed mid-chunk tn (softmax bias in ACT's Exp bias,
    denominator via a ones-matmul — gpsimd partition_all_reduce measured
    ~20us/iter SLOWER on HW; psdve=True keeps that variant — and the gate
    broadcast via ONE DMA roundtrip on the SP queue: per-DMA dispatch on the
    ACT queue costs ~664ns of ACT SEQ and would delay relus).
  - consts split early/late so gb(0)'s roundtrip isn't queued behind the
    bulk loads.
"""

import numpy as np

_P = 128          # partitions
_T = 4096         # tokens per core
_TN = 512         # token chunk (mm1 moving dim)
_NTN = _T // _TN  # 8
_HM = 32          # hid k-tiles (4096 / 128)
_E = 8
_OC = 256         # output channels
_NCORES = 8

_CACHE = {}


def _build_nc(reps=1, loop=1, stagger=False, ppk=2, lag=3, gpos=12,
              psdve=False):
    import concourse.bacc as bacc
    import concourse.bass as bass
    import concourse.bass_isa as bass_isa
    import concourse.mybir as mybir
    import concourse.tile as tile

    f32 = mybir.dt.float32
    bf16 = mybir.dt.bfloat16
    AF = mybir.ActivationFunctionType
    ts = bass.ts

    nc = bacc.Bacc("TRN2", target_bir_lowering=False, debug=False)

    xT_d = nc.dram_tensor("xT", [2, _NTN, _P, _TN], bf16, kind="ExternalInput").ap()
    w1_d = nc.dram_tensor("w1s", [2, 4, _P, 1024], bf16, kind="ExternalInput").ap()
    w2p_d = nc.dram_tensor("w2ps", [_P, _HM, _OC], bf16, kind="ExternalInput").ap()
    b1_d = nc.dram_tensor("b1s", [_P, _HM], f32, kind="ExternalInput").ap()
    wg_d = nc.dram_tensor("wgs", [2, _P, _E], bf16, kind="ExternalInput").ap()
    bg_d = nc.dram_tensor("bgs", [_E, 1], f32, kind="ExternalInput").ap()
    b2w_d = nc.dram_tensor("b2ws", [_E, _OC], bf16, kind="ExternalInput").ap()
    ones_d = nc.dram_tensor("ones", [_E, 1], bf16, kind="ExternalInput").ap()
    gst_d = nc.dram_tensor("gstage", [_NTN, _E, _TN], bf16).ap()
    out_d = nc.dram_tensor("out", [2, _P, _T], f32, kind="ExternalOutput").ap()

    with tile.TileContext(nc) as tc:
        with (
            tc.tile_pool(name="const", bufs=1) as const,
            tc.tile_pool(name="hs", bufs=2) as p_hs,
            tc.tile_pool(name="gb", bufs=2) as p_gb,
            tc.tile_pool(name="gsmall", bufs=2) as p_gs,
            tc.tile_pool(name="gaten", bufs=3) as p_gn,
            tc.tile_pool(name="ob", bufs=4) as p_ob,
            tc.tile_pool(name="psum_h", bufs=4, space="PSUM") as psum_h,
            tc.tile_pool(name="psum_o", bufs=3, space="PSUM") as psum_o,
            tc.tile_pool(name="psum_g", bufs=1, space="PSUM") as psum_g,
        ):
            xT_sb = const.tile([_P, 2, _T], bf16, name="xT_sb")
            w1_sb = const.tile([_P, 2, 4096], bf16, name="w1_sb")
            w2p_sb = const.tile([_P, _HM, _OC], bf16, name="w2p_sb")
            b1_sb = const.tile([_P, _HM], f32, name="b1_sb")
            wg_sb = const.tile([_P, 2, _E], bf16, name="wg_sb")
            bg_sb = const.tile([_E, 1], f32, name="bg_sb")
            b2w_sb = const.tile([_E, _OC], bf16, name="b2w_sb")
            ones_sb = const.tile([_E, 1], bf16, name="ones_sb")

            for kc in range(2):
                nc.sync.dma_start(out=wg_sb[:, kc, :], in_=wg_d[kc])
            nc.sync.dma_start(out=bg_sb[:], in_=bg_d[:])
            nc.sync.dma_start(out=ones_sb[:], in_=ones_d[:])
            # chunk-major DRAM layout: every chunk is one linear stream.
            # Only the consts chunk 0 needs are loaded here; the rest are
            # emitted by emit_late_consts AFTER gate(0), so gb(0)'s DMA
            # roundtrip isn't queued behind ~4MB of bulk loads.
            for kc in range(2):
                nc.sync.dma_start(out=xT_sb[:, kc, ts(0, _TN)], in_=xT_d[kc, 0])
            for kc in range(2):
                nc.sync.dma_start(out=w1_sb[:, kc, ts(0, 1024)], in_=w1_d[kc, 0])
            nc.sync.dma_start(out=b1_sb[:], in_=b1_d[:])
            nc.sync.dma_start(out=w2p_sb[:], in_=w2p_d[:])
            nc.sync.dma_start(out=b2w_sb[:], in_=b2w_d[:])

            def emit_late_consts():
                for q in range(1, 4):
                    for kc in range(2):
                        nc.sync.dma_start(out=w1_sb[:, kc, ts(q, 1024)],
                                          in_=w1_d[kc, q])
                for tn in range(1, _NTN):
                    for kc in range(2):
                        nc.sync.dma_start(out=xT_sb[:, kc, ts(tn, _TN)],
                                          in_=xT_d[kc, tn])

            def emit_gate(tn):
                """Gate for chunk tn: 3 PE ops + ACT/DVE/gpsimd chain + DMA
                broadcast roundtrip.  Called one chunk AHEAD of use so gb is
                ready when chunk tn's DVE muls need it."""
                pg = psum_g.tile([_E, _TN], f32, name="pg", tag="pg")
                nc.tensor.matmul(pg[:], wg_sb[:, 0, :], xT_sb[:, 0, ts(tn, _TN)],
                                 start=True, stop=False)
                nc.tensor.matmul(pg[:], wg_sb[:, 1, :], xT_sb[:, 1, ts(tn, _TN)],
                                 start=False, stop=True)
                expu = p_gs.tile([_E, _TN], bf16, name="expu", tag="expu")
                nc.scalar.activation(expu[:], pg[:], AF.Exp, bias=bg_sb[:])
                # softmax denominator: gpsimd partition all-reduce frees the
                # PE of a ones-matmul; then reciprocal + re-broadcast.
                rc = p_gs.tile([1, _TN], f32, name="rc", tag="rc")
                if psdve:
                    s8 = p_gs.tile([_E, _TN], f32, name="s8", tag="s8")
                    nc.gpsimd.partition_all_reduce(s8[:], expu[:], _E,
                                                   bass_isa.ReduceOp.add)
                    nc.vector.reciprocal(rc[:], s8[0:1, :])
                else:
                    ps = psum_g.tile([1, _TN], f32, name="ps", tag="pg")
                    nc.tensor.matmul(ps[:], ones_sb[:], expu[:],
                                     start=True, stop=True)
                    nc.vector.reciprocal(rc[:], ps[:])
                rcb = p_gs.tile([_E, _TN], f32, name="rcb", tag="rcb")
                nc.gpsimd.partition_broadcast(rcb[:], rc[:])
                gatenb = p_gn.tile([_E, _TN], bf16, name="gatenb", tag="gatenb")
                nc.vector.tensor_mul(gatenb[:], expu[:], rcb[:])
                # gpsimd partition_broadcast needs base partition 0: DMA the 8
                # gate rows to DRAM, then broadcast-read onto all partitions.
                gb = p_gb.tile([_P, _E, _TN], bf16, name="gb", tag="gb")
                nc.sync.dma_start(out=gst_d[tn], in_=gatenb[:])
                src_bc = gst_d[tn:tn + 1, :, :].broadcast_to((_P, _E, _TN))
                nc.sync.dma_start(out=gb[:, :, :], in_=src_bc)
                return gatenb, gb

            def emit_mm1_pair(tn, hm, hs, gb):
                ph = psum_h.tile([_P, _TN], f32, name="ph", tag="ph")
                nc.tensor.matmul(ph[:], w1_sb[:, 0, ts(hm, _P)],
                                 xT_sb[:, 0, ts(tn, _TN)], start=True, stop=False)
                nc.tensor.matmul(ph[:], w1_sb[:, 1, ts(hm, _P)],
                                 xT_sb[:, 1, ts(tn, _TN)], start=False, stop=True)
                # relu(+bias) straight into bf16 hs, then scale by gate in place.
                nc.scalar.activation(hs[:, hm, :], ph[:], AF.Relu,
                                     bias=b1_sb[:, hm:hm + 1])
                nc.vector.tensor_mul(hs[:, hm, :], hs[:, hm, :], gb[:, hm // 4, :])

            def finish_mm2(tn, po, oc):
                ob = p_ob.tile([_P, _TN], f32, name="ob", tag="ob")
                nc.scalar.copy(ob[:], po[:])
                nc.sync.dma_start(out=out_d[oc, :, ts(tn, _TN)], in_=ob[:])

            def mm2_steps(tn, hs, gatenb, mm2_pos):
                """Yield single mm2 PE ops for one chunk: w2p stationary,
                hs moving (N=512 tokens), out [128 oc, 512 tok].  kt-major
                with both oc-half accumulation groups open so each hs k-tile
                is consumed at exactly the rate mm1 produces them (2 mm2
                steps per mm1 pair)."""
                po = mm2_pos
                for kt in range(_HM):
                    for oc in range(2):
                        nc.tensor.matmul(po[oc][:], w2p_sb[:, kt, ts(oc, _P)],
                                         hs[:, kt, :],
                                         start=(kt == 0), stop=False)
                        yield
                for oc in range(2):
                    nc.tensor.matmul(po[oc][:], b2w_sb[:, ts(oc, _P)],
                                     gatenb[:], start=False, stop=True)
                    finish_mm2(tn, po[oc], oc)
                    yield

            def emit_body():
                # Same-chunk interleave: chunk tn's mm2 runs `lag` k-tiles
                # behind its mm1 (hs[kt] needs PE->ACT->DVE before mm2 reads
                # it).  The ~2*lag leftover mm2 steps drain during the next
                # chunk's first hms, so the PE stream is dense end to end and
                # the tail is ~2*lag matmuls instead of a whole chunk's mm2.
                # gate(tn+1) is emitted mid-chunk so its DMA-broadcast
                # roundtrip is done before chunk tn+1 starts.
                _DONE = object()

                def advance(gen, n):
                    for _ in range(n):
                        if next(gen, _DONE) is _DONE:
                            return None
                    return gen

                leftover = None
                gate_next = emit_gate(0)
                if emit_body.late_consts is not None:
                    emit_body.late_consts()
                    emit_body.late_consts = None
                for tn in range(_NTN):
                    gatenb, gb = gate_next
                    hs = p_hs.tile([_P, _HM, _TN], bf16, name="hs", tag="hs")
                    cur = None
                    cur_started = False
                    for hm in range(_HM):
                        emit_mm1_pair(tn, hm, hs, gb)
                        if hm == gpos and tn + 1 < _NTN:
                            gate_next = emit_gate(tn + 1)
                        if leftover is not None:
                            leftover = advance(leftover, ppk + 1)
                        elif hm >= lag and (cur is not None or not cur_started):
                            if cur is None:
                                pos = [psum_o.tile([_P, _TN], f32, name="po",
                                                   tag="po") for _ in range(2)]
                                cur = mm2_steps(tn, hs, gatenb, pos)
                                cur_started = True
                            cur = advance(cur, ppk)
                    assert leftover is None, "mm2 leftover not drained in-chunk"
                    if not cur_started:
                        pos = [psum_o.tile([_P, _TN], f32, name="po",
                                           tag="po") for _ in range(2)]
                        cur = mm2_steps(tn, hs, gatenb, pos)
                    leftover = cur
                if leftover is not None:
                    for _ in leftover:
                        pass

            emit_body.late_consts = emit_late_consts
            if loop > 1:
                # late consts must load outside the loop (once)
                emit_late_consts()
                emit_body.late_consts = None
                with tc.For_i(0, loop, 1, staggered_reset=stagger):
                    for _rep in range(reps):
                        emit_body()
            else:
                for _rep in range(reps):
                    emit_body()

    nc.compile()
    return nc


def _prep_weights(w1, b1, w2, b2, wg, bg, wo, bo):
    import ml_dtypes
    f32 = np.float32
    bf = ml_dtypes.bfloat16
    w1 = np.asarray(w1, f32)
    w2 = np.asarray(w2, f32)
    wo = np.asarray(wo, f32)
    E, IN, HID = w1.shape
    w1s = np.ascontiguousarray(
        w1.transpose(1, 0, 2).reshape(IN, E * HID).reshape(2, _P, 4, 1024)
        .transpose(0, 2, 1, 3)).astype(bf)
    w2p = (w2.astype(np.float64).reshape(E * HID, -1) @ wo.astype(np.float64)).astype(f32)
    w2ps = np.ascontiguousarray(
        w2p.reshape(_HM, _P, _OC).transpose(1, 0, 2)).astype(bf)
    b1s = np.ascontiguousarray(np.asarray(b1, f32).reshape(E * HID).reshape(_HM, _P).T)
    b2ws = (np.asarray(b2, np.float64) @ wo.astype(np.float64)
            + np.asarray(bo, np.float64)).astype(f32).astype(bf)
    wgs = np.ascontiguousarray(np.asarray(wg, f32).reshape(2, _P, E)).astype(bf)
    bgs = np.asarray(bg, f32).reshape(_E, 1)
    ones = np.ones((_E, 1), bf)
    return dict(w1s=w1s, w2ps=w2ps, b1s=b1s, b2ws=b2ws, wgs=wgs, bgs=bgs, ones=ones)


def _make_in_maps(x, w1, b1, w2, b2, wg, bg, wo, bo):
    import ml_dtypes
    bf = ml_dtypes.bfloat16
    x = np.asarray(x, np.float32)
    b, n, c = x.shape
    weights = _prep_weights(w1, b1, w2, b2, wg, bg, wo, bo)
    x2d = x.reshape(b * n, c)
    in_maps = []
    for i in range(_NCORES):
        xc = x2d[i * _T:(i + 1) * _T]                       # [T, C]
        xT = np.ascontiguousarray(
            xc.T.reshape(2, _P, _NTN, _TN).transpose(0, 2, 1, 3)).astype(bf)
        in_maps.append({"xT": xT, **weights})
    return in_maps


def _run(x, w1, b1, w2, b2, wg, bg, wo, bo, trace=False):
    from concourse.bass_utils import run_bass_kernel_spmd

    if "nc" not in _CACHE:
        _CACHE["nc"] = _build_nc(1)
    nc = _CACHE["nc"]

    x = np.asarray(x, np.float32)
    b, n, c = x.shape
    in_maps = _make_in_maps(x, w1, b1, w2, b2, wg, bg, wo, bo)
    res = run_bass_kernel_spmd(nc, in_maps, list(range(_NCORES)), trace=trace)
    # out_d is [oc_half, 128, T] per core: de-transpose to [T, OC] on host
    outs = []
    for i in range(_NCORES):
        o = res.results[i]["out"]                           # [2, 128, T]
        outs.append(o.reshape(_OC, _T).T)                   # [T, OC]
    out = np.concatenate(outs, axis=0)
    return np.ascontiguousarray(out).reshape(b, n, _OC), res


def kernel(x, w1, b1, w2, b2, wg, bg, wo, bo):
    out, _ = _run(x, w1, b1, w2, b2, wg, bg, wo, bo, trace=False)
    return out
